# revision 20
# baseline (speedup 1.0000x reference)
"""Trainium2 Bass kernel for a dense transformer self-attention block.

Problem: out = (softmax(QK^T/sqrt(dk) + mask) V) Wo + bo  with fused QKV proj.
  x [2, 2048, 1024], 16 heads, dk=64. Returns (out, attn).

Sharding (8 cores): core c handles batch b=c//4 and head-group g=c%4
(heads 4g..4g+3).  Pure data/tensor parallelism: no collectives; the
host sums the 4 output-projection partials per batch and reassembles
the attention probabilities (pure layout: transpose + dtype cast).

Device dataflow per core (all matmul operands bf16, accumulation f32):
  phase 1: qkT = [Wq'|Wk]^T x^T   (Wq' pre-scaled by 1/sqrt(dk) on host)
           V   = x Wv             (natural [k, dv] layout, + ones column)
  main, per (q-chunk, head-pair):
    scoresT[k,q] = K Q^T          (two heads packed in the PE array via
                                   row groups: dk=64 contraction each)
    E = exp(scoresT)              (ScalarE, straight from PSUM; max-
                                   subtraction skipped: |scores| <~ 30)
    blendT[dv+1, q] = [V|1]^T E   (ones column makes row 64 the softmax
                                   denominator - no partition reduce)
    r = 1/denom broadcast to 128 partitions (GpSimd)
    attn = E * r  -> HBM as [h, k, q] bf16 (host transposes to [q, k])
    blend = blendT * r
  out_partial[s, :] = blend^T Wo  -> HBM f32
"""

import os
import numpy as np
import ml_dtypes
from contextlib import ExitStack

B, S, D, H, DK = 2, 2048, 1024, 16, 64
NCORES = 8
HPC = 4  # heads per core
NEG_BIG = -float(2**63)
BF16 = ml_dtypes.bfloat16

PROFILE = False  # set True (e.g. from test.py) to neuron-profile the run
TRACE_DIR = None
LAST_EXEC_NS = None
LAST_RESULTS = None

_GRAPH_CACHE = {}


def _build_graph(use_mask, use_qk_bias, use_v_bias, debug_taps=False):
    import concourse.bass as bass
    import concourse.mybir as mybir
    import concourse.tile as tile
    from concourse import bacc

    DT = mybir.dt
    BF = DT.bfloat16
    F32 = DT.float32

    nc = bacc.Bacc("TRN2", target_bir_lowering=False, debug=False,
                   num_devices=NCORES)

    xt_d = nc.dram_tensor("xt", [D, S], BF, kind="ExternalInput")
    wqk_d = nc.dram_tensor("wqk", [D, 512], BF, kind="ExternalInput")
    wv_d = nc.dram_tensor("wv", [D, 256], BF, kind="ExternalInput")
    wo_d = nc.dram_tensor("wo", [256, D], BF, kind="ExternalInput")
    if use_mask:
        amb_d = nc.dram_tensor("amb", [128, 16], F32, kind="ExternalInput")
    if use_qk_bias:
        qkb_d = nc.dram_tensor("qkb", [128, 4], F32, kind="ExternalInput")
    if use_v_bias:
        vb_d = nc.dram_tensor("vb", [64, 4], F32, kind="ExternalInput")
    attn_d = nc.dram_tensor("attn_t", [HPC, S, S], BF, kind="ExternalOutput")
    outp_d = nc.dram_tensor("out_p", [S, D], F32, kind="ExternalOutput")
    if debug_taps:
        dbg_qkt = nc.dram_tensor("dbg_qkt", [4, 128, S], BF, kind="ExternalOutput")
        dbg_v = nc.dram_tensor("dbg_v", [128, 16, 264], BF, kind="ExternalOutput")
        dbg_exp = nc.dram_tensor("dbg_exp", [128, 16, 1024], BF,
                                 kind="ExternalOutput")
        dbg_den = nc.dram_tensor("dbg_den", [4, 512], F32, kind="ExternalOutput")
        dbg_rbc = nc.dram_tensor("dbg_rbc", [4, 128, 512], BF,
                                 kind="ExternalOutput")
        dbg_bl = nc.dram_tensor("dbg_bl", [4, 64, 512], BF, kind="ExternalOutput")

    EXP = mybir.ActivationFunctionType.Exp
    MUL = mybir.AluOpType.mult
    ADD = mybir.AluOpType.add

    with tile.TileContext(nc) as tc, ExitStack() as ctx:
        consts = ctx.enter_context(tc.tile_pool(name="consts", bufs=1))
        work = ctx.enter_context(tc.tile_pool(name="work", bufs=2))
        small = ctx.enter_context(tc.tile_pool(name="small", bufs=3))
        outs = ctx.enter_context(tc.tile_pool(name="outs", bufs=3))
        blpool = ctx.enter_context(tc.tile_pool(name="bl", bufs=8))
        # PSUM budget (8 banks): scores 2x2 + blend 2x1 + mm1 2x1 = 8
        ps_s = ctx.enter_context(tc.tile_pool(name="ps_s", bufs=2, space="PSUM"))
        ps_b = ctx.enter_context(tc.tile_pool(name="ps_b", bufs=2, space="PSUM"))
        ps_m = ctx.enter_context(tc.tile_pool(name="ps_m", bufs=2, space="PSUM"))

        # ---- constant loads -------------------------------------------------
        wqk_sb = consts.tile([128, 8, 512], BF, tag="wqk")
        nc.sync.dma_start(wqk_sb[:], wqk_d.ap().rearrange("(o p) m -> p o m", p=128))
        wv_sb = consts.tile([128, 8, 256], BF, tag="wv")
        nc.sync.dma_start(wv_sb[:], wv_d.ap().rearrange("(o p) m -> p o m", p=128))
        # head-major [64, 4, 1024]: out-proj runs as 4 accumulating K=64
        # matmuls so per-head blend tiles can stay at partition base 0
        wo_sb = consts.tile([64, 4, 1024], BF, tag="wo")
        nc.sync.dma_start(wo_sb[:], wo_d.ap().rearrange("(h p) m -> p h m", p=64))
        xt_sb = []
        for dc in range(8):
            t = consts.tile([128, S], BF, tag=f"xt{dc}")
            nc.sync.dma_start(t[:], xt_d.ap()[dc * 128:(dc + 1) * 128, :])
            xt_sb.append(t)
        if use_mask:
            amb_sb = consts.tile([128, 16], F32, tag="amb")
            nc.sync.dma_start(amb_sb[:], amb_d.ap())
        if use_qk_bias:
            qkb_sb = consts.tile([128, 4], F32, tag="qkb")
            nc.sync.dma_start(qkb_sb[:], qkb_d.ap())
        if use_v_bias:
            vb_sb = consts.tile([64, 4], F32, tag="vb")
            nc.sync.dma_start(vb_sb[:], vb_d.ap())

        qT_sb = consts.tile([128, 2, S], BF, tag="qT")
        kT_sb = consts.tile([128, 2, S], BF, tag="kT")
        # v_aug layout: per head a 66-col strip: [V(64) | ones | pad]
        v_sb = consts.tile([128, 16, 264], BF, tag="v")
        nc.vector.memset(v_sb[:], 1.0)
        ones_sb = consts.tile([128, 128], BF, tag="ones")
        nc.vector.memset(ones_sb[:], 1.0)

        # ---- phase 1: qkT = [Wq'|Wk]^T @ x^T  ([m, s], m-tiles = head pairs)
        for mt in range(4):
            target = qT_sb if mt < 2 else kT_sb
            pr = mt % 2
            for sc in range(4):
                ps = ps_m.tile([128, 512], F32, tag="mm1")
                for dc in range(8):
                    nc.tensor.matmul(
                        ps[:],
                        lhsT=wqk_sb[:, dc, mt * 128:(mt + 1) * 128],
                        rhs=xt_sb[dc][:, sc * 512:(sc + 1) * 512],
                        start=(dc == 0), stop=(dc == 7))
                dst = target[:, pr, sc * 512:(sc + 1) * 512]
                if use_qk_bias:
                    nc.scalar.activation(
                        out=dst, in_=ps[:],
                        func=mybir.ActivationFunctionType.Identity,
                        bias=qkb_sb[:, mt:mt + 1])
                else:
                    nc.scalar.copy(out=dst, in_=ps[:])

        # ---- phase 1: V = x @ Wv  (natural [s, dv] layout)
        for st in range(16):
            ps = ps_m.tile([128, 256], F32, tag="mm1")
            for dc in range(8):
                nc.tensor.matmul(
                    ps[:],
                    lhsT=xt_sb[dc][:, st * 128:(st + 1) * 128],
                    rhs=wv_sb[:, dc, :],
                    start=(dc == 0), stop=(dc == 7))
            dst = v_sb[:, st, :].rearrange("p (h c) -> p h c", h=4)[:, :, 0:64]
            src = ps[:].rearrange("p (h c) -> p h c", h=4)
            nc.scalar.copy(out=dst, in_=src)
        if debug_taps:
            for mt in range(4):
                tgt = qT_sb if mt < 2 else kT_sb
                nc.sync.dma_start(out=dbg_qkt.ap()[mt, :, :],
                                  in_=tgt[:, mt % 2, :])
            nc.sync.dma_start(out=dbg_v.ap()[:, :, :], in_=v_sb[:])

        # ---- main loop ------------------------------------------------------
        for qc in range(4):
            blend_h = [None] * 4
            for pr in range(2):
                attn_un = work.tile([128, 16, 1024], BF, tag="attn_un")
                for kc in range(16):
                    ps = ps_s.tile([128, 1024], F32, tag="scores")
                    for m in range(2):
                        nc.tensor.matmul(
                            ps[:, m * 512:(m + 1) * 512],
                            lhsT=kT_sb[m * 64:(m + 1) * 64, pr,
                                       kc * 128:(kc + 1) * 128],
                            rhs=qT_sb[m * 64:(m + 1) * 64, pr,
                                      qc * 512:(qc + 1) * 512],
                            start=True, stop=True)
                    bias = amb_sb[:, kc:kc + 1] if use_mask else 0.0
                    nc.scalar.activation(out=attn_un[:, kc, :], in_=ps[:],
                                         func=EXP, bias=bias)
                if debug_taps and qc == 0 and pr == 0:
                    nc.sync.dma_start(out=dbg_exp.ap()[:, :, :], in_=attn_un[:])
                for m in range(2):
                    h = 2 * pr + m
                    psb = ps_b.tile([128, 512], F32, tag="blend_ps")
                    for kc in range(16):
                        nc.tensor.matmul(
                            psb[0:65, :],
                            lhsT=v_sb[:, kc, h * 66:h * 66 + 65],
                            rhs=attn_un[:, kc, m * 512:(m + 1) * 512],
                            start=(kc == 0), stop=(kc == 15))
                    # denominator (psum row 64) -> bf16 -> PE ones-matmul
                    # broadcast to all 128 partitions -> reciprocal at base 0
                    # (reciprocal_approx_fast is broken at non-zero base)
                    rb = small.tile([128, 512], BF, tag="rb")
                    nc.vector.tensor_copy(out=rb[64:65, :], in_=psb[64:65, :])
                    rbp = ps_b.tile([128, 512], F32, tag="blend_ps")
                    nc.tensor.matmul(rbp[:, :], lhsT=ones_sb[64:65, 0:128],
                                     rhs=rb[64:65, :], start=True, stop=True)
                    rc = small.tile([128, 512], F32, tag="rc")
                    nc.vector.reciprocal_approx_fast(out=rc[:, :],
                                                     in_=rbp[:, :])
                    rbc = small.tile([128, 512], BF, tag="rbc")
                    nc.vector.tensor_copy(out=rbc[:, :], in_=rc[:, :])
                    if debug_taps and qc == 0:
                        nc.sync.dma_start(out=dbg_den.ap()[h:h + 1, :],
                                          in_=rc[0:1, :])
                        nc.sync.dma_start(out=dbg_rbc.ap()[h, :, :],
                                          in_=rbc[:, :])
                    # normalize attn tiles in place, then write out
                    sl = attn_un[:, :, m * 512:(m + 1) * 512]
                    nc.vector.tensor_tensor(
                        out=sl, in0=sl,
                        in1=rbc[:, None, :].to_broadcast((128, 16, 512)),
                        op=MUL)
                    nc.sync.dma_start(
                        out=attn_d.ap()[h].rearrange("(t p) q -> p t q", p=128)
                        [:, :, qc * 512:(qc + 1) * 512],
                        in_=sl)
                    # normalize blended values (f32 from PSUM, -> bf16)
                    bt = blpool.tile([64, 512], BF, tag="blendh")
                    nc.vector.tensor_tensor(out=bt[:], in0=psb[0:64, :],
                                            in1=rbc[0:64, :], op=MUL)
                    if use_v_bias:
                        nc.vector.tensor_scalar(
                            out=bt[:], in0=bt[:], scalar1=vb_sb[:, h:h + 1],
                            scalar2=None, op0=ADD)
                    blend_h[h] = bt
                    if debug_taps and qc == 0:
                        nc.sync.dma_start(out=dbg_bl.ap()[h, :, :], in_=bt[:])

            # ---- output projection for this q-chunk (s == q) ----------------
            for st in range(4):
                s0 = qc * 512 + st * 128
                out_sb = outs.tile([128, 1024], F32, tag="out")
                for nk in range(2):
                    pso = ps_m.tile([128, 512], F32, tag="mm1")
                    for h in range(4):
                        nc.tensor.matmul(
                            pso[:],
                            lhsT=blend_h[h][:, st * 128:(st + 1) * 128],
                            rhs=wo_sb[:, h, nk * 512:(nk + 1) * 512],
                            start=(h == 0), stop=(h == 3))
                    nc.vector.tensor_copy(out=out_sb[:, nk * 512:(nk + 1) * 512],
                                          in_=pso[:])
                nc.sync.dma_start(out=outp_d.ap()[s0:s0 + 128, :], in_=out_sb[:])

    nc.compile()
    return nc


def _prep_inputs(x, attention_mask, wk_w, wk_b, wo_w):
    """Shard + lay out the full inputs for the 8 cores (host-side numpy)."""
    x = np.asarray(x, np.float32)
    wk_w = np.asarray(wk_w, np.float32)
    wk_b = np.asarray(wk_b, np.float32)
    wo_w = np.asarray(wo_w, np.float32)
    am = np.asarray(attention_mask, np.float32)

    amb = 1.0 - am.reshape(B, S)
    amb = np.where(amb == 1.0, np.float32(NEG_BIG), amb).astype(np.float32)
    use_mask = bool(np.any(amb != 0.0))
    use_qk_bias = bool(np.any(wk_b[:2 * D] != 0.0))
    use_v_bias = bool(np.any(wk_b[2 * D:] != 0.0))

    scale = np.float32(1.0 / np.sqrt(DK))
    xt = [np.ascontiguousarray(x[b].T).astype(BF16) for b in range(B)]

    in_maps = []
    for c in range(NCORES):
        b, g = divmod(c, 4)
        cols = slice(g * 256, (g + 1) * 256)
        wq = wk_w[:, cols] * scale
        wk = wk_w[:, 1024:2048][:, cols]
        wv = wk_w[:, 2048:3072][:, cols]
        m = {
            "xt": xt[b],
            "wqk": np.ascontiguousarray(
                np.concatenate([wq, wk], axis=1)).astype(BF16),
            "wv": np.ascontiguousarray(wv).astype(BF16),
            "wo": np.ascontiguousarray(wo_w[g * 256:(g + 1) * 256, :]).astype(BF16),
        }
        if use_mask:
            m["amb"] = np.ascontiguousarray(
                amb[b].reshape(16, 128).T).astype(np.float32)
        if use_qk_bias:
            qb = wk_b[cols] * scale
            kb = wk_b[1024:2048][cols]
            m["qkb"] = np.ascontiguousarray(np.stack(
                [qb[:128], qb[128:], kb[:128], kb[128:]], axis=1)).astype(np.float32)
        if use_v_bias:
            vb = wk_b[2048:3072][cols]
            m["vb"] = np.ascontiguousarray(
                vb.reshape(4, 64).T).astype(np.float32)
        in_maps.append(m)
    return in_maps, (use_mask, use_qk_bias, use_v_bias)


def kernel(x, attention_mask, wk_w, wk_b, wo_w, wo_b):
    global LAST_EXEC_NS, LAST_RESULTS
    from concourse.bass_utils import run_bass_kernel_spmd

    in_maps, variant = _prep_inputs(x, attention_mask, wk_w, wk_b, wo_w)
    if variant not in _GRAPH_CACHE:
        _GRAPH_CACHE[variant] = _build_graph(*variant)
    nc = _GRAPH_CACHE[variant]

    kw = {}
    if PROFILE and TRACE_DIR:
        kw["tmpdir"] = TRACE_DIR
    res = run_bass_kernel_spmd(nc, in_maps, core_ids=list(range(NCORES)),
                               trace=PROFILE, **kw)
    LAST_EXEC_NS = res.exec_time_ns
    results = res.results
    LAST_RESULTS = results

    out = np.zeros([B, S, D], np.float32)
    attn = np.empty([B, H, S, S], np.float32)
    for c in range(NCORES):
        b, g = divmod(c, 4)
        out[b] += np.asarray(results[c]["out_p"], np.float32)
        at = np.asarray(results[c]["attn_t"])  # [4, k, q] bf16
        attn[b, g * 4:(g + 1) * 4] = at.transpose(0, 2, 1).astype(np.float32)
    wo_b = np.asarray(wo_b, np.float32)
    if np.any(wo_b != 0.0):
        out += wo_b
    return out, attn
